# revision 26
# baseline (speedup 1.0000x reference)
"""CRD loss kernel for 8 Trainium2 NeuronCores (v8).

Math notes (derived from the CRDLoss reference):
  - neg_scores gathers student rows idx[i,j] = j + (j>=i) which only ever
    touches student rows 0..10 ("head"); the rest of the student projection
    (and all logits / contrast_idx / idx inputs) are dead.
  - log(exp(u)+c) = u + c*exp(-u) + O(c^2 e^-2u) with c = m/M + EPS ~ 1.8e-4
    and u = s/T in ~N(0, 1.26^2): the quadratic term contributes ~3e-8
    relative loss error.  So slog = sum(u) (plain DVE reduce) + c*sum(e^-u)
    (one Exp activation with the hw accumulator).  Scalar only ever runs
    Exp/Identity -> exactly one ACT_TABLE_LOAD, warmed at kernel start.
  - 1/sqrt on the Vector engine: 0x5f3759df bit hack (in f32 value space,
    no int ops) + one Newton iteration (~0.2% worst case, ~1e-5 loss err).

Layout per core (rows sharded 2048/core):
  - anchor features x: [128(k), combo, block, kt, 512(r)] fp8; combo 0
    lands in 512-row block chunks, combos 1..3 whole, all on the sync
    queue in consumption order.
  - projection y^T accumulates in PSUM [128,512] per block via fp8
    DoubleRow matmuls (2 k-tiles per instruction, 2x rate); bias-add +
    bf16 convert runs on Scalar (Identity, per-partition bias).
  - scores: per 128-anchor chunk, matmul lhsT=yb[:,chunk] (bf16) with
    rhs=[h_hat0..h_hat10] gives [128 anchors, 11] scores; lhsT=sq chunk
    with a ones column gives the norm^2 in col 11.  Anchors on partitions
    means rsq broadcasts with a stride-0 AP, the shifted-head correction
    (rows 0..10 of core 0) is a free AP column offset, and the reduce /
    accumulators directly yield the per-core partial sums.
  - emission order software-pipelines: proj(q0), heads, proj(q1),
    score(q0)+tail(q0), proj(q2), score(q1)+tail(q1), ...
"""

import sys

for _p in ("/opt/trn_rl_repo", "/root/.axon_site/_ro/trn_rl_repo"):
    if _p not in sys.path:
        sys.path.insert(0, _p)

import math
import os

import ml_dtypes
import numpy as np

import concourse.bass as bass  # noqa: F401
import concourse.tile as tile
from concourse import bacc, mybir
from concourse.bass_utils import run_bass_kernel_spmd

F32 = mybir.dt.float32
F32R = mybir.dt.float32r
BF16 = mybir.dt.bfloat16
FP8 = mybir.dt.float8e4
U32 = mybir.dt.uint32
WSCALE = 64.0
AF = mybir.ActivationFunctionType
ALU = mybir.AluOpType

EPS = 1e-07
K = 10
T = 0.07
DIN = 1024
DOUT = 128
N = 16384
NCORES = 8
SH = N // NCORES          # 2048 rows per core
NKT = DIN // 128          # 8 k-tiles
BLK = 512
NBLK = SH // BLK          # 4 row blocks per core
CH = 128                  # anchors per score chunk
NCHB = BLK // CH          # 4 chunks per block
NCH = SH // CH            # 16 chunks per combo
NHH = 11                  # head columns used (h_hat 0..10)
SC = NHH + 1              # psum cols per chunk (11 scores + 1 norm^2)
NH = 16                   # head rows shipped

# (anchor feature, anchor W, anchor b, side); side 0 = entity student head.
COMBOS = [
    ("entity_features_TeaE", "We_tE", "be_tE", 0),
    ("entity_features_TeaR", "We_tR", "be_tR", 0),
    ("rel_features_TeaE", "Wr_tE", "br_tE", 1),
    ("rel_features_TeaR", "Wr_tR", "br_tR", 1),
]
HEADS = [("entity_features_s", "We_s", "be_s"), ("rel_features_s", "Wr_s", "br_s")]

MAGIC = 0x5F3759DF
USE_DR = os.environ.get("K_DR", "1") == "1"
USE_GPS_MUL = os.environ.get("K_GPSMUL", "0") == "1"
USE_TTR = os.environ.get("K_TTR", "0") == "1"
YB_SCALAR = os.environ.get("K_YBSC", "1") == "1"

_CACHE = {}


def _newton_rsqrt(nc, pool, v, n, final_scale=1.0, p=128):
    """r = final_scale / sqrt(v) on DVE, no activation tables.

    One Newton iteration after the bit hack: <=0.18% rel error, which is
    ~1e-5 relative on the final loss (errors are random across anchors).
    """
    r0 = pool.tile([p, n], F32, tag="nw_r0")
    t = pool.tile([p, n], F32, tag="nw_t")
    r1 = pool.tile([p, n], F32, tag="nw_r1")
    # r0_bits = MAGIC - v_bits/2, in f32 value arithmetic (the +-bit
    # rounding is noise vs the hack's own error).  The dtype-mismatched
    # read/write does the u32<->f32 value conversion.
    nc.vector.tensor_scalar(
        out=t[:], in0=v.bitcast(U32),
        scalar1=-0.5, scalar2=float(MAGIC),
        op0=ALU.mult, op1=ALU.add,
    )
    nc.vector.tensor_scalar(
        out=r0.bitcast(U32), in0=t[:], scalar1=0.0, scalar2=None, op0=ALU.add,
    )
    # r1 = r0 * fs * (1.5 - 0.5 v r0^2)
    nc.vector.tensor_mul(out=t[:], in0=r0[:], in1=r0[:])
    nc.vector.tensor_mul(out=t[:], in0=t[:], in1=v[:])
    nc.vector.tensor_scalar(out=t[:], in0=t[:],
                            scalar1=-0.5 * final_scale,
                            scalar2=1.5 * final_scale,
                            op0=ALU.mult, op1=ALU.add)
    nc.vector.tensor_mul(out=r1[:], in0=r0[:], in1=t[:])
    return r1


def _build(c_const):
    """Build + compile the SPMD program. c_const = m*Pn + EPS."""
    nc = bacc.Bacc("TRN2", target_bir_lowering=False, debug=False)

    xdr = nc.dram_tensor("x", [128, 4, NBLK, NKT, BLK], FP8, kind="ExternalInput")
    wdr = nc.dram_tensor("w", [128, 4, NKT, DOUT], FP8, kind="ExternalInput")
    bdr = nc.dram_tensor("b", [DOUT, 4], F32, kind="ExternalInput")
    whdr = nc.dram_tensor("wh", [128, 2, NKT, DOUT], BF16, kind="ExternalInput")
    hdr = nc.dram_tensor("h", [128, 2, NKT, NH], BF16, kind="ExternalInput")
    bhdr = nc.dram_tensor("bh", [DOUT, 2], F32, kind="ExternalInput")
    fbdr = nc.dram_tensor("fb", [128, 20], F32, kind="ExternalInput")
    outdr = nc.dram_tensor("out", [128, 12], F32, kind="ExternalOutput")

    with tile.TileContext(nc) as tc:
        with (
            tc.tile_pool(name="consts", bufs=1) as consts,
            tc.tile_pool(name="xp", bufs=1) as xp,
            tc.tile_pool(name="ybp", bufs=2) as ybp,
            tc.tile_pool(name="sqp", bufs=2) as sqp,
            tc.tile_pool(name="stp", bufs=2) as stp,
            tc.tile_pool(name="tiny", bufs=4) as tinyp,
            tc.tile_pool(name="pacc", bufs=3, space="PSUM") as pacc,
            tc.tile_pool(name="pjunk", bufs=1, space="PSUM") as pjunk,
            tc.tile_pool(name="psco", bufs=2, space="PSUM") as psco,
            tc.tile_pool(name="ptiny", bufs=2, space="PSUM") as ptiny,
        ):
            # ---- tiles ----
            x_t = xp.tile([128, 4, NBLK, NKT, BLK], FP8, tag="x")
            w_t = consts.tile([128, 4, NKT, DOUT], FP8, tag="w")
            b_t = consts.tile([DOUT, 4], F32, tag="b")
            wh_t = consts.tile([128, 2, NKT, DOUT], BF16, tag="wh")
            h_t = consts.tile([128, 2, NKT, NH], BF16, tag="h")
            bh_t = consts.tile([DOUT, 2], F32, tag="bh")
            fb_t = consts.tile([128, 20], F32, tag="fb")
            ones_knr = consts.tile([128, NH], F32, tag="ones_knr")
            ones_1p = consts.tile([1, 128], F32, tag="ones_1p")
            onebf = consts.tile([128, 1], F32, tag="onebf")
            hh = [consts.tile([128, 16], BF16, name=f"hh{s}", tag=f"hh{s}")
                  for s in range(2)]
            acc_t = consts.tile([128, 12], F32, tag="acc")

            # ---- DMA issue: x on sync (consumption order), consts on gpsimd
            for q in range(4):
                for blk in range(NBLK):
                    eng = nc.sync if (4 * q + blk) % 2 == 0 else nc.scalar
                    eng.dma_start(out=x_t[:, q, blk], in_=xdr[:, q, blk])
            nc.gpsimd.dma_start(out=w_t[:], in_=wdr[:])
            nc.gpsimd.dma_start(out=wh_t[:], in_=whdr[:])
            nc.gpsimd.dma_start(out=h_t[:], in_=hdr[:])
            nc.gpsimd.dma_start(out=b_t[:], in_=bdr[:])
            nc.gpsimd.dma_start(out=bh_t[:], in_=bhdr[:])
            nc.gpsimd.dma_start(out=fb_t[:], in_=fbdr[:])

            nc.vector.memset(ones_knr[:], 1.0)
            nc.vector.memset(ones_1p[:], 1.0)
            nc.vector.memset(onebf[:], 1.0)

            # warm the exp table while DMAs stream
            spw = tinyp.tile([1, 1], F32, tag="spw")
            nc.vector.memset(spw[:], 0.0)
            nc.scalar.activation(out=spw[:], in_=spw[:], func=AF.Exp)

            # dependency-free matmuls keep the PE clock at full p-state
            # while the tensor queue waits on x DMAs
            jw = consts.tile([128, 256], BF16, tag="jw")
            nc.vector.memset(jw[:], 0.0)
            jp = pjunk.tile([128, 256], F32, tag="jp")

            def emit_warm(n):
                for _ in range(n):
                    nc.tensor.matmul(out=jp[:], lhsT=jw[:, 0:128], rhs=jw[:],
                                     start=True, stop=True,
                                     skip_group_check=True)

            yb_ts, sq_ts, sco_pss = {}, {}, {}

            def emit_proj(q):
                yb_t = ybp.tile([128, NBLK, BLK], BF16, tag="yb")
                sq_t = sqp.tile([128, NBLK, BLK], BF16, tag="sq")
                yb_ts[q], sq_ts[q] = yb_t, sq_t
                acc_list = []
                for blk in range(NBLK):
                    acc_ps = pacc.tile([128, BLK], F32, tag="pacc")
                    if USE_DR:
                        for kt in range(0, NKT, 2):
                            nc.tensor.matmul(
                                out=acc_ps[:],
                                lhsT=w_t[:, q, kt:kt + 2, :],
                                rhs=x_t[:, q, blk, kt:kt + 2, :],
                                start=(kt == 0),
                                stop=(kt == NKT - 2),
                                perf_mode=mybir.MatmulPerfMode.DoubleRow,
                            )
                    else:
                        for kt in range(NKT):
                            nc.tensor.matmul(
                                out=acc_ps[:],
                                lhsT=w_t[:, q, kt, :],
                                rhs=x_t[:, q, blk, kt, :],
                                start=(kt == 0),
                                stop=(kt == NKT - 1),
                            )
                    acc_list.append(acc_ps)
                sq_eng = nc.gpsimd if USE_GPS_MUL else nc.vector
                for blk in range(NBLK):
                    if YB_SCALAR:
                        nc.scalar.activation(out=yb_t[:, blk],
                                             in_=acc_list[blk][:],
                                             func=AF.Identity,
                                             bias=b_t[:, q:q + 1])
                    else:
                        nc.vector.tensor_scalar_add(out=yb_t[:, blk],
                                                    in0=acc_list[blk][:],
                                                    scalar1=b_t[:, q:q + 1])
                    sq_eng.tensor_mul(out=sq_t[:, blk], in0=yb_t[:, blk],
                                      in1=yb_t[:, blk])

            def emit_heads():
                for s in range(2):
                    yh_ps = ptiny.tile([128, NH], F32, name="yh_ps", tag="ptiny")
                    for kt in range(NKT):
                        nc.tensor.matmul(
                            out=yh_ps[:],
                            lhsT=wh_t[:, s, kt, :],
                            rhs=h_t[:, s, kt, :],
                            start=(kt == 0),
                            stop=(kt == NKT - 1),
                        )
                    yh = tinyp.tile([128, NH], F32, name=f"yh{s}", tag=f"yh{s}")
                    nc.vector.tensor_scalar_add(out=yh[:], in0=yh_ps[:],
                                                scalar1=bh_t[:, s:s + 1])
                    sqh = tinyp.tile([128, NH], F32R, name="sqh", tag="sqh")
                    nc.vector.tensor_mul(out=sqh[:], in0=yh[:], in1=yh[:])
                    nsq_ps = ptiny.tile([NH, NH], F32, name="nsqh_ps", tag="ptiny")
                    nc.tensor.matmul(out=nsq_ps[:],
                                     lhsT=ones_knr[:].bitcast(F32R),
                                     rhs=sqh[:], start=True, stop=True)
                    nsqh = tinyp.tile([1, NH], F32, name="nsqh", tag="nsqh")
                    nc.vector.tensor_copy(out=nsqh[:], in_=nsq_ps[0:1, :])
                    rsqh = _newton_rsqrt(nc, tinyp, nsqh, NH, p=1)
                    rsqh_r = tinyp.tile([1, NH], F32R, name="rsqh_r",
                                        tag="rsqh_r")
                    nc.vector.tensor_copy(out=rsqh_r[:], in_=rsqh[:])
                    rsqb_ps = ptiny.tile([128, NH], F32, name="rsqb_ps",
                                         tag="ptiny")
                    nc.tensor.matmul(out=rsqb_ps[:],
                                     lhsT=ones_1p[:].bitcast(F32R),
                                     rhs=rsqh_r[:], start=True, stop=True)
                    nc.vector.tensor_mul(out=hh[s][:, 0:NHH], in0=yh[:, 0:NHH],
                                         in1=rsqb_ps[:, 0:NHH])
                    nc.vector.tensor_copy(out=hh[s][:, NHH:NHH + 1],
                                          in_=onebf[:])

            def emit_score(q):
                s = COMBOS[q][3]
                yb_t, sq_t = yb_ts[q], sq_ts[q]
                sco_ps = psco.tile([128, NCH, SC], F32, tag="psco")
                sco_pss[q] = sco_ps
                for blk in range(NBLK):
                    for j in range(NCHB):
                        c = NCHB * blk + j
                        cs = slice(CH * j, CH * j + CH)
                        nc.tensor.matmul(
                            out=sco_ps[:, c, 0:NHH],
                            lhsT=yb_t[:, blk, cs],
                            rhs=hh[s][:, 0:NHH],
                            start=True, stop=True,
                        )
                        nc.tensor.matmul(
                            out=sco_ps[:, c, NHH:SC],
                            lhsT=sq_t[:, blk, cs],
                            rhs=hh[s][:, NHH:NHH + 1],
                            start=True, stop=True,
                        )

            def emit_tail(q):
                sco_ps = sco_pss[q]
                v = stp.tile([128, NCH], F32, tag="v")
                nc.vector.tensor_copy(out=v[:], in_=sco_ps[:, :, NHH])
                rsq = _newton_rsqrt(nc, stp, v, NCH, final_scale=1.0 / T)
                # st = u for cols 0..9 (and col 10 of chunk 0 for the shift)
                st_t = stp.tile([128, NCH, SC], F32, tag="st")
                if USE_TTR:
                    nc.vector.tensor_tensor_reduce(
                        out=st_t[:, :, 0:K],
                        in0=sco_ps[:, :, 0:K],
                        in1=rsq[:].unsqueeze(2).broadcast_to((128, NCH, K)),
                        scale=1.0, scalar=0.0,
                        op0=ALU.mult, op1=ALU.add,
                        accum_out=acc_t[:, 4 + q:5 + q],
                    )
                else:
                    nc.vector.tensor_mul(
                        out=st_t[:, :, 0:K],
                        in0=sco_ps[:, :, 0:K],
                        in1=rsq[:].unsqueeze(2).broadcast_to((128, NCH, K)),
                    )
                    ur = stp.tile([128, NCH], F32, tag="ur")
                    nc.vector.reduce_sum(out=ur[:], in_=st_t[:, :, 0:K],
                                         axis=mybir.AxisListType.X)
                    nc.vector.reduce_sum(out=acc_t[:, 4 + q:5 + q],
                                         in_=ur[:],
                                         axis=mybir.AxisListType.X)
                nc.vector.tensor_mul(out=st_t[:, 0, K:NHH],
                                     in0=sco_ps[:, 0, K:NHH],
                                     in1=rsq[:, 0:1])
                # local rows 0..9 shifted-head correction (flag gates core 0)
                d = tinyp.tile([16, 16], F32, tag="d")
                nc.vector.tensor_sub(out=d[0:10, 0:10],
                                     in0=st_t[0:10, 0, 1:11],
                                     in1=st_t[0:10, 0, 0:10])
                nc.vector.tensor_mul(out=d[0:10, 0:10], in0=d[0:10, 0:10],
                                     in1=fb_t[0:10, 0:10])
                nc.vector.tensor_scalar_mul(out=d[0:10, 0:10],
                                            in0=d[0:10, 0:10],
                                            scalar1=fb_t[0:10, 16:17])
                nc.vector.tensor_add(out=st_t[0:10, 0, 0:10],
                                     in0=st_t[0:10, 0, 0:10],
                                     in1=d[0:10, 0:10])
                dr = tinyp.tile([16, 1], F32, tag="dr")
                nc.vector.reduce_sum(out=dr[0:10, :], in_=d[0:10, 0:10],
                                     axis=mybir.AxisListType.X)
                nc.vector.tensor_add(out=acc_t[0:10, 4 + q:5 + q],
                                     in0=acc_t[0:10, 4 + q:5 + q],
                                     in1=dr[0:10, :])
                # c * sum(exp(-u)) correction term -> acc col q
                sp_scr = stp.tile([128, NCH, K], BF16, tag="spscr")
                nc.scalar.activation(out=sp_scr[:], in_=st_t[:, :, 0:K],
                                     func=AF.Exp, scale=-1.0,
                                     accum_out=acc_t[:, q:q + 1])
                # spos partial: col 0 -> acc col 8+q
                nc.vector.reduce_sum(out=acc_t[:, 8 + q:9 + q],
                                     in_=st_t[:, :, 0],
                                     axis=mybir.AxisListType.X)

            # ---- software-pipelined emission ----
            emit_warm(40)
            emit_proj(0)
            emit_heads()
            emit_warm(6)
            emit_proj(1)
            emit_score(0)
            emit_tail(0)
            emit_warm(6)
            emit_proj(2)
            emit_score(1)
            emit_tail(1)
            emit_warm(6)
            emit_proj(3)
            emit_score(2)
            emit_tail(2)
            emit_score(3)
            emit_tail(3)

            nc.sync.dma_start(out=outdr[:], in_=acc_t[:])

    nc.compile()
    return nc


def _pack_x(feat):
    """[B,TS,DIN] f32 -> per-core [128, NBLK, NKT, BLK] fp8, k-major."""
    f = np.ascontiguousarray(np.asarray(feat, dtype=np.float32)).reshape(N, DIN)
    # (core, blk, r, kt, p) -> (core, p, blk, kt, r)
    v = f.reshape(NCORES, NBLK, BLK, NKT, 128).transpose(0, 4, 1, 3, 2)
    return np.ascontiguousarray(v.astype(ml_dtypes.float8_e4m3))


def _pack_w8(w):
    # x64 lands typical N(0, 0.02^2) weights in the fp8 normal range; the
    # scale cancels in the L2 normalization (biases scaled to match).
    v = (np.asarray(w, dtype=np.float32) * WSCALE).reshape(NKT, 128, DOUT)
    return np.ascontiguousarray(v.transpose(1, 0, 2).astype(ml_dtypes.float8_e4m3))


def _pack_wh(w):
    v = np.asarray(w, dtype=np.float32).reshape(NKT, 128, DOUT).transpose(1, 0, 2)
    return np.ascontiguousarray(v.astype(ml_dtypes.bfloat16))


def _pack_h(feat):
    f = np.asarray(feat, dtype=np.float32).reshape(N, DIN)[0:NH]  # [16, 1024]
    v = f.T.reshape(NKT, 128, NH).transpose(1, 0, 2)
    return np.ascontiguousarray(v.astype(ml_dtypes.bfloat16))


def kernel(**inputs):
    M = int(np.asarray(inputs["M"]))
    m = K - 1
    Pn = 1.0 / float(M)
    c_const = m * Pn + EPS

    key = ("v8", M)
    if key not in _CACHE:
        _CACHE[key] = _build(c_const)
    nc = _CACHE[key]

    xs = np.stack([_pack_x(inputs[COMBOS[q][0]]) for q in range(4)], axis=2)
    w = np.stack([_pack_w8(inputs[COMBOS[q][1]]) for q in range(4)], axis=1)
    b = np.stack(
        [np.asarray(inputs[COMBOS[q][2]], np.float32) * WSCALE for q in range(4)],
        axis=1,
    ).astype(np.float32)
    wh = np.stack([_pack_wh(inputs[HEADS[s][1]]) for s in range(2)], axis=1)
    h = np.stack([_pack_h(inputs[HEADS[s][0]]) for s in range(2)], axis=1)
    bh = np.stack(
        [np.asarray(inputs[HEADS[s][2]], np.float32) for s in range(2)], axis=1
    ).astype(np.float32)

    j = np.arange(16)[None, :]
    i = np.arange(16)[:, None]
    fb = np.zeros((128, 20), np.float32)
    fb[0:16, 0:16] = (j >= i).astype(np.float32)

    in_maps = []
    for cid in range(NCORES):
        fbc = fb.copy()
        fbc[:, 16] = 1.0 if cid == 0 else 0.0
        im = {"x": xs[cid], "w": w, "b": np.ascontiguousarray(b),
              "wh": wh, "h": h, "bh": np.ascontiguousarray(bh), "fb": fbc}
        in_maps.append(im)

    res = run_bass_kernel_spmd(nc, in_maps, list(range(NCORES)))
    global LAST_RESULT
    LAST_RESULT = res

    outs = np.stack([np.asarray(res.results[cid]["out"]).astype(np.float64)
                     for cid in range(NCORES)])  # [8, 128, 12]
    sume = outs[:, :, 0:4].sum(axis=(0, 1))   # sum exp(-u) per combo
    sumu = outs[:, :, 4:8].sum(axis=(0, 1))   # sum u per combo
    spos = outs[:, :, 8:12].sum(axis=(0, 1))  # pos-score sums (already / T)
    slog = sumu + c_const * sume              # sum log(exp(u)+c)
    const = 9.0 * N * math.log(m * Pn)
    loss = -(spos + const - slog) / N  # [4]
    return np.array([loss[0] + loss[1], loss[2] + loss[3]], dtype=np.float32)


if __name__ == "__main__":
    rng = np.random.default_rng(0)
    fake = {}
    for nm in ("entity_features_s", "rel_features_s", "entity_features_TeaE",
               "rel_features_TeaE", "entity_features_TeaR", "rel_features_TeaR"):
        fake[nm] = rng.standard_normal((16, 1024, DIN), dtype=np.float32)
    for nm in ("entity_logits_TeaE", "rel_logits_TeaE", "entity_logits_TeaR",
               "rel_logits_TeaR"):
        fake[nm] = rng.standard_normal((16, 1024, 100), dtype=np.float32)
    for pn in ("We_s", "We_tE", "We_tR", "Wr_s", "Wr_tE", "Wr_tR"):
        fake[pn] = (rng.standard_normal((DIN, DOUT), dtype=np.float32) * 0.02)
        fake[pn.replace("W", "b", 1)] = np.zeros((DOUT,), np.float32)
    fake["contrast_idx"] = rng.integers(0, 50000, size=(N,))
    fake["idx"] = rng.integers(0, 50000, size=(N,))
    fake["M"] = 50000
    print(kernel(**fake))


# revision 28
# speedup vs baseline: 1.0262x; 1.0262x over previous
"""CRD loss kernel for 8 Trainium2 NeuronCores (v8).

Math notes (derived from the CRDLoss reference):
  - neg_scores gathers student rows idx[i,j] = j + (j>=i) which only ever
    touches student rows 0..10 ("head"); the rest of the student projection
    (and all logits / contrast_idx / idx inputs) are dead.
  - log(exp(u)+c) = u + c*exp(-u) + O(c^2 e^-2u) with c = m/M + EPS ~ 1.8e-4
    and u = s/T in ~N(0, 1.26^2): the quadratic term contributes ~3e-8
    relative loss error.  So slog = sum(u) (plain DVE reduce) + c*sum(e^-u)
    (one Exp activation with the hw accumulator).  Scalar only ever runs
    Exp/Identity -> exactly one ACT_TABLE_LOAD, warmed at kernel start.
  - 1/sqrt on the Vector engine: 0x5f3759df bit hack (in f32 value space,
    no int ops) + one Newton iteration (~0.2% worst case, ~1e-5 loss err).

Layout per core (rows sharded 2048/core):
  - anchor features x: [128(k), combo, block, kt, 512(r)] fp8; combo 0
    lands in 512-row block chunks, combos 1..3 whole, all on the sync
    queue in consumption order.
  - projection y^T accumulates in PSUM [128,512] per block via fp8
    DoubleRow matmuls (2 k-tiles per instruction, 2x rate); bias-add +
    bf16 convert runs on Scalar (Identity, per-partition bias).
  - scores: per 128-anchor chunk, matmul lhsT=yb[:,chunk] (bf16) with
    rhs=[h_hat0..h_hat10] gives [128 anchors, 11] scores; lhsT=sq chunk
    with a ones column gives the norm^2 in col 11.  Anchors on partitions
    means rsq broadcasts with a stride-0 AP, the shifted-head correction
    (rows 0..10 of core 0) is a free AP column offset, and the reduce /
    accumulators directly yield the per-core partial sums.
  - emission order software-pipelines: proj(q0), heads, proj(q1),
    score(q0)+tail(q0), proj(q2), score(q1)+tail(q1), ...
"""

import sys

for _p in ("/opt/trn_rl_repo", "/root/.axon_site/_ro/trn_rl_repo"):
    if _p not in sys.path:
        sys.path.insert(0, _p)

import math
import os

import ml_dtypes
import numpy as np

import concourse.bass as bass  # noqa: F401
import concourse.tile as tile
from concourse import bacc, mybir
from concourse.bass_utils import run_bass_kernel_spmd

F32 = mybir.dt.float32
F32R = mybir.dt.float32r
BF16 = mybir.dt.bfloat16
FP8 = mybir.dt.float8e4
U32 = mybir.dt.uint32
WSCALE = 64.0
AF = mybir.ActivationFunctionType
ALU = mybir.AluOpType

EPS = 1e-07
K = 10
T = 0.07
DIN = 1024
DOUT = 128
N = 16384
NCORES = 8
SH = N // NCORES          # 2048 rows per core
NKT = DIN // 128          # 8 k-tiles
BLK = 512
NBLK = SH // BLK          # 4 row blocks per core
CH = 128                  # anchors per score chunk
NCHB = BLK // CH          # 4 chunks per block
NCH = SH // CH            # 16 chunks per combo
NHH = 11                  # head columns used (h_hat 0..10)
SC = NHH + 1              # psum cols per chunk (11 scores + 1 norm^2)
NH = 16                   # head rows shipped

# (anchor feature, anchor W, anchor b, side); side 0 = entity student head.
COMBOS = [
    ("entity_features_TeaE", "We_tE", "be_tE", 0),
    ("entity_features_TeaR", "We_tR", "be_tR", 0),
    ("rel_features_TeaE", "Wr_tE", "br_tE", 1),
    ("rel_features_TeaR", "Wr_tR", "br_tR", 1),
]
HEADS = [("entity_features_s", "We_s", "be_s"), ("rel_features_s", "Wr_s", "br_s")]

MAGIC = 0x5F3759DF
USE_DR = os.environ.get("K_DR", "1") == "1"
USE_GPS_MUL = os.environ.get("K_GPSMUL", "0") == "1"
USE_TTR = os.environ.get("K_TTR", "0") == "1"
YB_SCALAR = os.environ.get("K_YBSC", "1") == "1"

_CACHE = {}


def _newton_rsqrt(nc, pool, v, n, final_scale=1.0, p=128):
    """r = final_scale / sqrt(v) on DVE, no activation tables.

    One Newton iteration after the bit hack: <=0.18% rel error, which is
    ~1e-5 relative on the final loss (errors are random across anchors).
    """
    r0 = pool.tile([p, n], F32, tag="nw_r0")
    t = pool.tile([p, n], F32, tag="nw_t")
    r1 = pool.tile([p, n], F32, tag="nw_r1")
    # r0_bits = MAGIC - v_bits/2, in f32 value arithmetic (the +-bit
    # rounding is noise vs the hack's own error).  The dtype-mismatched
    # read/write does the u32<->f32 value conversion.
    nc.vector.tensor_scalar(
        out=t[:], in0=v.bitcast(U32),
        scalar1=-0.5, scalar2=float(MAGIC),
        op0=ALU.mult, op1=ALU.add,
    )
    nc.vector.tensor_scalar(
        out=r0.bitcast(U32), in0=t[:], scalar1=0.0, scalar2=None, op0=ALU.add,
    )
    # r1 = r0 * fs * (1.5 - 0.5 v r0^2)
    nc.vector.tensor_mul(out=t[:], in0=r0[:], in1=r0[:])
    nc.vector.tensor_mul(out=t[:], in0=t[:], in1=v[:])
    nc.vector.tensor_scalar(out=t[:], in0=t[:],
                            scalar1=-0.5 * final_scale,
                            scalar2=1.5 * final_scale,
                            op0=ALU.mult, op1=ALU.add)
    nc.vector.tensor_mul(out=r1[:], in0=r0[:], in1=t[:])
    return r1


def _build(c_const):
    """Build + compile the SPMD program. c_const = m*Pn + EPS."""
    nc = bacc.Bacc("TRN2", target_bir_lowering=False, debug=False)

    xdr = nc.dram_tensor("x", [128, 4, NBLK, NKT, BLK], FP8, kind="ExternalInput")
    wdr = nc.dram_tensor("w", [128, 4, NKT, DOUT], FP8, kind="ExternalInput")
    bdr = nc.dram_tensor("b", [DOUT, 4], F32, kind="ExternalInput")
    whdr = nc.dram_tensor("wh", [128, 2, NKT, DOUT], BF16, kind="ExternalInput")
    hdr = nc.dram_tensor("h", [128, 2, NKT, NH], BF16, kind="ExternalInput")
    bhdr = nc.dram_tensor("bh", [DOUT, 2], F32, kind="ExternalInput")
    fbdr = nc.dram_tensor("fb", [128, 20], F32, kind="ExternalInput")
    outdr = nc.dram_tensor("out", [128, 12], F32, kind="ExternalOutput")

    with tile.TileContext(nc) as tc:
        with (
            tc.tile_pool(name="consts", bufs=1) as consts,
            tc.tile_pool(name="xp", bufs=1) as xp,
            tc.tile_pool(name="ybp", bufs=2) as ybp,
            tc.tile_pool(name="sqp", bufs=2) as sqp,
            tc.tile_pool(name="stp", bufs=2) as stp,
            tc.tile_pool(name="tiny", bufs=4) as tinyp,
            tc.tile_pool(name="pacc", bufs=3, space="PSUM") as pacc,
            tc.tile_pool(name="pjunk", bufs=1, space="PSUM") as pjunk,
            tc.tile_pool(name="psco", bufs=2, space="PSUM") as psco,
            tc.tile_pool(name="ptiny", bufs=2, space="PSUM") as ptiny,
        ):
            # ---- tiles ----
            x_t = xp.tile([128, 4, NBLK, NKT, BLK], FP8, tag="x")
            w_t = consts.tile([128, 4, NKT, DOUT], FP8, tag="w")
            b_t = consts.tile([DOUT, 4], F32, tag="b")
            wh_t = consts.tile([128, 2, NKT, DOUT], BF16, tag="wh")
            h_t = consts.tile([128, 2, NKT, NH], BF16, tag="h")
            bh_t = consts.tile([DOUT, 2], F32, tag="bh")
            fb_t = consts.tile([128, 20], F32, tag="fb")
            ones_knr = consts.tile([128, NH], F32, tag="ones_knr")
            ones_1p = consts.tile([1, 128], F32, tag="ones_1p")
            onebf = consts.tile([128, 1], F32, tag="onebf")
            hh = [consts.tile([128, 16], BF16, name=f"hh{s}", tag=f"hh{s}")
                  for s in range(2)]
            acc_t = consts.tile([128, 12], F32, tag="acc")

            # ---- DMA issue: x on sync (consumption order), consts on gpsimd
            for q in range(4):
                for blk in range(NBLK):
                    nc.sync.dma_start(out=x_t[:, q, blk], in_=xdr[:, q, blk])
            nc.gpsimd.dma_start(out=w_t[:], in_=wdr[:])
            nc.gpsimd.dma_start(out=wh_t[:], in_=whdr[:])
            nc.gpsimd.dma_start(out=h_t[:], in_=hdr[:])
            nc.gpsimd.dma_start(out=b_t[:], in_=bdr[:])
            nc.gpsimd.dma_start(out=bh_t[:], in_=bhdr[:])
            nc.gpsimd.dma_start(out=fb_t[:], in_=fbdr[:])

            nc.vector.memset(ones_knr[:], 1.0)
            nc.vector.memset(ones_1p[:], 1.0)
            nc.vector.memset(onebf[:], 1.0)

            # warm the exp table while DMAs stream
            spw = tinyp.tile([1, 1], F32, tag="spw")
            nc.vector.memset(spw[:], 0.0)
            nc.scalar.activation(out=spw[:], in_=spw[:], func=AF.Exp)

            # dependency-free matmuls keep the PE clock at full p-state
            # while the tensor queue waits on x DMAs
            jw = consts.tile([128, 256], BF16, tag="jw")
            nc.vector.memset(jw[:], 0.0)
            jp = pjunk.tile([128, 256], F32, tag="jp")

            def emit_warm(n):
                for _ in range(n):
                    nc.tensor.matmul(out=jp[:], lhsT=jw[:, 0:128], rhs=jw[:],
                                     start=True, stop=True,
                                     skip_group_check=True)

            yb_ts, sq_ts, sco_pss = {}, {}, {}

            def emit_proj(q):
                yb_t = ybp.tile([128, NBLK, BLK], BF16, tag="yb")
                sq_t = sqp.tile([128, NBLK, BLK], BF16, tag="sq")
                yb_ts[q], sq_ts[q] = yb_t, sq_t
                acc_list = []
                for blk in range(NBLK):
                    acc_ps = pacc.tile([128, BLK], F32, tag="pacc")
                    if USE_DR:
                        for kt in range(0, NKT, 2):
                            nc.tensor.matmul(
                                out=acc_ps[:],
                                lhsT=w_t[:, q, kt:kt + 2, :],
                                rhs=x_t[:, q, blk, kt:kt + 2, :],
                                start=(kt == 0),
                                stop=(kt == NKT - 2),
                                perf_mode=mybir.MatmulPerfMode.DoubleRow,
                            )
                    else:
                        for kt in range(NKT):
                            nc.tensor.matmul(
                                out=acc_ps[:],
                                lhsT=w_t[:, q, kt, :],
                                rhs=x_t[:, q, blk, kt, :],
                                start=(kt == 0),
                                stop=(kt == NKT - 1),
                            )
                    acc_list.append(acc_ps)
                sq_eng = nc.gpsimd if USE_GPS_MUL else nc.vector
                for blk in range(NBLK):
                    if YB_SCALAR:
                        nc.scalar.activation(out=yb_t[:, blk],
                                             in_=acc_list[blk][:],
                                             func=AF.Identity,
                                             bias=b_t[:, q:q + 1])
                    else:
                        nc.vector.tensor_scalar_add(out=yb_t[:, blk],
                                                    in0=acc_list[blk][:],
                                                    scalar1=b_t[:, q:q + 1])
                    sq_eng.tensor_mul(out=sq_t[:, blk], in0=yb_t[:, blk],
                                      in1=yb_t[:, blk])

            def emit_heads():
                for s in range(2):
                    yh_ps = ptiny.tile([128, NH], F32, name="yh_ps", tag="ptiny")
                    for kt in range(NKT):
                        nc.tensor.matmul(
                            out=yh_ps[:],
                            lhsT=wh_t[:, s, kt, :],
                            rhs=h_t[:, s, kt, :],
                            start=(kt == 0),
                            stop=(kt == NKT - 1),
                        )
                    yh = tinyp.tile([128, NH], F32, name=f"yh{s}", tag=f"yh{s}")
                    nc.vector.tensor_scalar_add(out=yh[:], in0=yh_ps[:],
                                                scalar1=bh_t[:, s:s + 1])
                    sqh = tinyp.tile([128, NH], F32R, name="sqh", tag="sqh")
                    nc.vector.tensor_mul(out=sqh[:], in0=yh[:], in1=yh[:])
                    nsq_ps = ptiny.tile([NH, NH], F32, name="nsqh_ps", tag="ptiny")
                    nc.tensor.matmul(out=nsq_ps[:],
                                     lhsT=ones_knr[:].bitcast(F32R),
                                     rhs=sqh[:], start=True, stop=True)
                    nsqh = tinyp.tile([1, NH], F32, name="nsqh", tag="nsqh")
                    nc.vector.tensor_copy(out=nsqh[:], in_=nsq_ps[0:1, :])
                    rsqh = _newton_rsqrt(nc, tinyp, nsqh, NH, p=1)
                    rsqh_r = tinyp.tile([1, NH], F32R, name="rsqh_r",
                                        tag="rsqh_r")
                    nc.vector.tensor_copy(out=rsqh_r[:], in_=rsqh[:])
                    rsqb_ps = ptiny.tile([128, NH], F32, name="rsqb_ps",
                                         tag="ptiny")
                    nc.tensor.matmul(out=rsqb_ps[:],
                                     lhsT=ones_1p[:].bitcast(F32R),
                                     rhs=rsqh_r[:], start=True, stop=True)
                    nc.vector.tensor_mul(out=hh[s][:, 0:NHH], in0=yh[:, 0:NHH],
                                         in1=rsqb_ps[:, 0:NHH])
                    nc.vector.tensor_copy(out=hh[s][:, NHH:NHH + 1],
                                          in_=onebf[:])

            def emit_score(q):
                s = COMBOS[q][3]
                yb_t, sq_t = yb_ts[q], sq_ts[q]
                sco_ps = psco.tile([128, NCH, SC], F32, tag="psco")
                sco_pss[q] = sco_ps
                for blk in range(NBLK):
                    for j in range(NCHB):
                        c = NCHB * blk + j
                        cs = slice(CH * j, CH * j + CH)
                        nc.tensor.matmul(
                            out=sco_ps[:, c, 0:NHH],
                            lhsT=yb_t[:, blk, cs],
                            rhs=hh[s][:, 0:NHH],
                            start=True, stop=True,
                        )
                        nc.tensor.matmul(
                            out=sco_ps[:, c, NHH:SC],
                            lhsT=sq_t[:, blk, cs],
                            rhs=hh[s][:, NHH:NHH + 1],
                            start=True, stop=True,
                        )

            def emit_tail(q):
                sco_ps = sco_pss[q]
                v = stp.tile([128, NCH], F32, tag="v")
                nc.vector.tensor_copy(out=v[:], in_=sco_ps[:, :, NHH])
                rsq = _newton_rsqrt(nc, stp, v, NCH, final_scale=1.0 / T)
                # st = u for cols 0..9 (and col 10 of chunk 0 for the shift)
                st_t = stp.tile([128, NCH, SC], F32, tag="st")
                if USE_TTR:
                    nc.vector.tensor_tensor_reduce(
                        out=st_t[:, :, 0:K],
                        in0=sco_ps[:, :, 0:K],
                        in1=rsq[:].unsqueeze(2).broadcast_to((128, NCH, K)),
                        scale=1.0, scalar=0.0,
                        op0=ALU.mult, op1=ALU.add,
                        accum_out=acc_t[:, 4 + q:5 + q],
                    )
                else:
                    nc.vector.tensor_mul(
                        out=st_t[:, :, 0:K],
                        in0=sco_ps[:, :, 0:K],
                        in1=rsq[:].unsqueeze(2).broadcast_to((128, NCH, K)),
                    )
                    ur = stp.tile([128, NCH], F32, tag="ur")
                    nc.vector.reduce_sum(out=ur[:], in_=st_t[:, :, 0:K],
                                         axis=mybir.AxisListType.X)
                    nc.vector.reduce_sum(out=acc_t[:, 4 + q:5 + q],
                                         in_=ur[:],
                                         axis=mybir.AxisListType.X)
                nc.vector.tensor_mul(out=st_t[:, 0, K:NHH],
                                     in0=sco_ps[:, 0, K:NHH],
                                     in1=rsq[:, 0:1])
                # local rows 0..9 shifted-head correction (flag gates core 0)
                d = tinyp.tile([16, 16], F32, tag="d")
                nc.vector.tensor_sub(out=d[0:10, 0:10],
                                     in0=st_t[0:10, 0, 1:11],
                                     in1=st_t[0:10, 0, 0:10])
                nc.vector.tensor_mul(out=d[0:10, 0:10], in0=d[0:10, 0:10],
                                     in1=fb_t[0:10, 0:10])
                nc.vector.tensor_scalar_mul(out=d[0:10, 0:10],
                                            in0=d[0:10, 0:10],
                                            scalar1=fb_t[0:10, 16:17])
                nc.vector.tensor_add(out=st_t[0:10, 0, 0:10],
                                     in0=st_t[0:10, 0, 0:10],
                                     in1=d[0:10, 0:10])
                dr = tinyp.tile([16, 1], F32, tag="dr")
                nc.vector.reduce_sum(out=dr[0:10, :], in_=d[0:10, 0:10],
                                     axis=mybir.AxisListType.X)
                nc.vector.tensor_add(out=acc_t[0:10, 4 + q:5 + q],
                                     in0=acc_t[0:10, 4 + q:5 + q],
                                     in1=dr[0:10, :])
                # c * sum(exp(-u)) correction term -> acc col q
                sp_scr = stp.tile([128, NCH, K], BF16, tag="spscr")
                nc.scalar.activation(out=sp_scr[:], in_=st_t[:, :, 0:K],
                                     func=AF.Exp, scale=-1.0,
                                     accum_out=acc_t[:, q:q + 1])
                # spos partial: col 0 -> acc col 8+q
                nc.vector.reduce_sum(out=acc_t[:, 8 + q:9 + q],
                                     in_=st_t[:, :, 0],
                                     axis=mybir.AxisListType.X)

            # ---- software-pipelined emission ----
            emit_warm(40)
            emit_proj(0)
            emit_heads()
            emit_warm(6)
            emit_proj(1)
            emit_score(0)
            emit_tail(0)
            emit_warm(6)
            emit_proj(2)
            emit_score(1)
            emit_tail(1)
            emit_warm(6)
            emit_proj(3)
            emit_score(2)
            emit_tail(2)
            emit_score(3)
            emit_tail(3)

            out_eng = nc.gpsimd if os.environ.get("K_OUTGPS", "1") == "1" else nc.sync
            out_eng.dma_start(out=outdr[:], in_=acc_t[:])

    nc.compile()
    return nc


def _pack_x(feat):
    """[B,TS,DIN] f32 -> per-core [128, NBLK, NKT, BLK] fp8, k-major."""
    f = np.ascontiguousarray(np.asarray(feat, dtype=np.float32)).reshape(N, DIN)
    # (core, blk, r, kt, p) -> (core, p, blk, kt, r)
    v = f.reshape(NCORES, NBLK, BLK, NKT, 128).transpose(0, 4, 1, 3, 2)
    return np.ascontiguousarray(v.astype(ml_dtypes.float8_e4m3))


def _pack_w8(w):
    # x64 lands typical N(0, 0.02^2) weights in the fp8 normal range; the
    # scale cancels in the L2 normalization (biases scaled to match).
    v = (np.asarray(w, dtype=np.float32) * WSCALE).reshape(NKT, 128, DOUT)
    return np.ascontiguousarray(v.transpose(1, 0, 2).astype(ml_dtypes.float8_e4m3))


def _pack_wh(w):
    v = np.asarray(w, dtype=np.float32).reshape(NKT, 128, DOUT).transpose(1, 0, 2)
    return np.ascontiguousarray(v.astype(ml_dtypes.bfloat16))


def _pack_h(feat):
    f = np.asarray(feat, dtype=np.float32).reshape(N, DIN)[0:NH]  # [16, 1024]
    v = f.T.reshape(NKT, 128, NH).transpose(1, 0, 2)
    return np.ascontiguousarray(v.astype(ml_dtypes.bfloat16))


def kernel(**inputs):
    M = int(np.asarray(inputs["M"]))
    m = K - 1
    Pn = 1.0 / float(M)
    c_const = m * Pn + EPS

    key = ("v8", M)
    if key not in _CACHE:
        _CACHE[key] = _build(c_const)
    nc = _CACHE[key]

    xs = np.stack([_pack_x(inputs[COMBOS[q][0]]) for q in range(4)], axis=2)
    w = np.stack([_pack_w8(inputs[COMBOS[q][1]]) for q in range(4)], axis=1)
    b = np.stack(
        [np.asarray(inputs[COMBOS[q][2]], np.float32) * WSCALE for q in range(4)],
        axis=1,
    ).astype(np.float32)
    wh = np.stack([_pack_wh(inputs[HEADS[s][1]]) for s in range(2)], axis=1)
    h = np.stack([_pack_h(inputs[HEADS[s][0]]) for s in range(2)], axis=1)
    bh = np.stack(
        [np.asarray(inputs[HEADS[s][2]], np.float32) for s in range(2)], axis=1
    ).astype(np.float32)

    j = np.arange(16)[None, :]
    i = np.arange(16)[:, None]
    fb = np.zeros((128, 20), np.float32)
    fb[0:16, 0:16] = (j >= i).astype(np.float32)

    in_maps = []
    for cid in range(NCORES):
        fbc = fb.copy()
        fbc[:, 16] = 1.0 if cid == 0 else 0.0
        im = {"x": xs[cid], "w": w, "b": np.ascontiguousarray(b),
              "wh": wh, "h": h, "bh": np.ascontiguousarray(bh), "fb": fbc}
        in_maps.append(im)

    res = run_bass_kernel_spmd(nc, in_maps, list(range(NCORES)))
    global LAST_RESULT
    LAST_RESULT = res

    outs = np.stack([np.asarray(res.results[cid]["out"]).astype(np.float64)
                     for cid in range(NCORES)])  # [8, 128, 12]
    sume = outs[:, :, 0:4].sum(axis=(0, 1))   # sum exp(-u) per combo
    sumu = outs[:, :, 4:8].sum(axis=(0, 1))   # sum u per combo
    spos = outs[:, :, 8:12].sum(axis=(0, 1))  # pos-score sums (already / T)
    slog = sumu + c_const * sume              # sum log(exp(u)+c)
    const = 9.0 * N * math.log(m * Pn)
    loss = -(spos + const - slog) / N  # [4]
    return np.array([loss[0] + loss[1], loss[2] + loss[3]], dtype=np.float32)


if __name__ == "__main__":
    rng = np.random.default_rng(0)
    fake = {}
    for nm in ("entity_features_s", "rel_features_s", "entity_features_TeaE",
               "rel_features_TeaE", "entity_features_TeaR", "rel_features_TeaR"):
        fake[nm] = rng.standard_normal((16, 1024, DIN), dtype=np.float32)
    for nm in ("entity_logits_TeaE", "rel_logits_TeaE", "entity_logits_TeaR",
               "rel_logits_TeaR"):
        fake[nm] = rng.standard_normal((16, 1024, 100), dtype=np.float32)
    for pn in ("We_s", "We_tE", "We_tR", "Wr_s", "Wr_tE", "Wr_tR"):
        fake[pn] = (rng.standard_normal((DIN, DOUT), dtype=np.float32) * 0.02)
        fake[pn.replace("W", "b", 1)] = np.zeros((DOUT,), np.float32)
    fake["contrast_idx"] = rng.integers(0, 50000, size=(N,))
    fake["idx"] = rng.integers(0, 50000, size=(N,))
    fake["M"] = 50000
    print(kernel(**fake))
